# revision 19
# baseline (speedup 1.0000x reference)
"""Trainium2 Bass kernel for fused causal GQA attention block.

Reference computation (B=1, S=2048, H=4096, NH=32, NKV=8, HD=128):
    qkv = hs @ w_attn.T; rope(q), rope(k); causal GQA attention;
    out @ w_proj.T

Sharding (8 cores, tensor parallel): core i owns kv-group i = rows
[i*768, (i+1)*768) of w_attn (4 q heads + 1 k + 1 v head) and rows
[i*512, (i+1)*512) of w_proj.

All heavy compute runs in bf16 (fp32 PSUM accumulation): full-rate PE
with fast weight load, half the DMA/SBUF/collective traffic of fp32.

Schedule: for each 512-seq block nb: hs resident in SBUF; QKV GEMM in
two weight-group passes with k/v computed FIRST so the V transposes
(XBAR DMA) and k-rope are ready well before any collective can starve
the SDMA engines -> rope(q) -> attention chunk nb (4 q-blocks of 128,
all 4 heads fused into the 512-wide free dim sharing K/V; causal mask
added into PSUM by an extra matmul; exp on ACT software-pipelined 2
deep against the score matmuls; softmax denominator via a ones-matmul,
fast approximate reciprocal on DVE). Each block's bf16 AllGather fires
after the NEXT block's V transposes to dodge XBAR/collective
contention; a tiny warmup AllGather absorbs first-collective latency
and inter-core launch skew. c_proj consumes gathered chunks at the
end, covering the last AllGather. DMA queues are specialized (sync:
hs/vT/at/wp/lh/y HWDGE; scalar: wa + exp; gpsimd: consts + wa half +
collectives) to avoid FIFO head-of-line blocking across streams with
different dependency depths.
"""

import sys

sys.path.insert(0, "/opt/trn_rl_repo")

import ml_dtypes
import numpy as np

import concourse.bass as bass
import concourse.tile as tile
from concourse import bacc, mybir
from concourse.bass_utils import run_bass_kernel_spmd

F32 = mybir.dt.float32
BF16 = mybir.dt.bfloat16
BF16NP = ml_dtypes.bfloat16

B, S, H = 1, 2048, 4096
NH, NKV, HD = 32, 8, 128
GROUP = NH // NKV  # 4
SCALE = 0.08838834764831845
NCORES = 8

M_SHARD = (GROUP + 2) * HD  # 768 rows of w_attn per core
P_SHARD = H // NCORES  # 512 rows of w_proj per core

KC = H // 128  # 32 contraction chunks of the model dim
NB = S // 512  # 4 seq blocks of 512
MT = M_SHARD // 128  # 6 row tiles of qkv_t
QT = S // 128  # 16 q blocks of 128
MASKBIG = -600.0  # additive causal mask (-600 * SCALE ~ -53 before exp)


def build_module() -> bass.Bass:
    nc = bacc.Bacc(
        "TRN2",
        target_bir_lowering=False,
        debug=False,
        num_devices=NCORES,
    )

    hs_t = nc.dram_tensor("hs_t", [H, S], BF16, kind="ExternalInput")
    wa_t = nc.dram_tensor("wa_t", [H, M_SHARD], BF16, kind="ExternalInput")
    wp_t = nc.dram_tensor("wp_t", [H, P_SHARD], BF16, kind="ExternalInput")
    cos_t = nc.dram_tensor("cos_t", [HD, S], BF16, kind="ExternalInput")
    sin_t = nc.dram_tensor("sin_t", [HD, S], BF16, kind="ExternalInput")
    rot_t = nc.dram_tensor("rot_t", [HD, HD], BF16, kind="ExternalInput")
    masks_in = nc.dram_tensor("masks_in", [128, 512], BF16, kind="ExternalInput")
    ones_in = nc.dram_tensor("ones_in", [128, 128], BF16, kind="ExternalInput")
    ident_in = nc.dram_tensor("ident_in", [128, 128], BF16, kind="ExternalInput")
    y_out = nc.dram_tensor("y", [S, P_SHARD], F32, kind="ExternalOutput")

    warm_in = nc.dram_tensor("warm_in", [1, 64], BF16, kind="Internal")
    warm_out = nc.dram_tensor(
        "warm_out", [8, 64], BF16, kind="Internal", addr_space="Shared"
    )
    # per-seq-chunk collective buffers (bf16 halves the wire bytes)
    ag_ins = [
        nc.dram_tensor(f"ag_in{i}", [GROUP * HD, 512], BF16, kind="Internal")
        for i in range(NB)
    ]
    ag_outs = [
        nc.dram_tensor(
            f"ag_out{i}", [H, 512], BF16, kind="Internal", addr_space="Shared"
        )
        for i in range(NB)
    ]

    # DRAM views with 128-partition tiling of the contraction axis
    hs_v = hs_t[:].rearrange("(ko p) n -> p ko n", p=128)  # [128, 32, 2048]
    wa_v = wa_t[:].rearrange("(ko p) m -> p ko m", p=128)  # [128, 32, 768]
    wp_v = wp_t[:].rearrange("(ko p) m -> p ko m", p=128)  # [128, 32, 512]
    ag_rd = [a[:].rearrange("(ko p) n -> p ko n", p=128) for a in ag_outs]
    # write view: feature row h*128+d <- at[d (part), (h, qq)]
    ag_wr = [a[:].rearrange("(h d) s -> d h s", h=GROUP) for a in ag_ins]

    with tile.TileContext(nc) as tc:
        # ---------- persistent pools ----------
        qkv_pool = tc.alloc_tile_pool(name="qkv", bufs=1)
        w_pool = tc.alloc_tile_pool(name="w", bufs=1)
        const_pool = tc.alloc_tile_pool(name="consts", bufs=1)
        vnat_pool = tc.alloc_tile_pool(name="vnat", bufs=1)
        rope_pool = tc.alloc_tile_pool(name="rope", bufs=2)
        pt_pool = tc.alloc_tile_pool(name="pt", bufs=4)
        attn_pool = tc.alloc_tile_pool(name="attn", bufs=3)
        psST = tc.alloc_tile_pool(name="psST", bufs=3, space="PSUM")
        psLO = tc.alloc_tile_pool(name="psLO", bufs=1, space="PSUM")
        hs_pool = tc.alloc_tile_pool(name="hs", bufs=2)
        psA = tc.alloc_tile_pool(name="psA", bufs=1, space="PSUM")

        qkv_sb = qkv_pool.tile([128, MT, S], BF16)  # 24KB/part
        wa_sb = w_pool.tile([128, KC, M_SHARD], BF16)  # 48KB/part
        v_nat = vnat_pool.tile([128, QT, HD], BF16)  # 4KB/part

        ones_sb = const_pool.tile([128, 128], BF16, tag="ones")
        ident_sb = const_pool.tile([128, 128], BF16, tag="ident")
        rot_sb = const_pool.tile([128, HD], BF16, tag="rot")
        masks_sb = const_pool.tile([128, 512], BF16, tag="masks")
        cos_sb = const_pool.tile([128, S], BF16, tag="cos")
        sin_sb = const_pool.tile([128, S], BF16, tag="sin")

        # ---------- preloads ----------
        # wa group-0 (k/v/q3 = cols 384:768) fine-split on the fast scalar
        # path; group-1 (cols 0:384) early on gpsimd
        mc0, mc1 = slice(384, 768), slice(0, 384)
        for lo, hi in zip(
            [0, 1, 2, 4, 8, 16, 24], [1, 2, 4, 8, 16, 24, 32]
        ):
            nc.scalar.dma_start(
                out=wa_sb[:, lo:hi, mc0], in_=wa_v[:, lo:hi, mc0]
            )
        for lo, hi in zip([0, 8, 16, 24], [8, 16, 24, 32]):
            nc.gpsimd.dma_start(
                out=wa_sb[:, lo:hi, mc1], in_=wa_v[:, lo:hi, mc1]
            )
        nc.gpsimd.dma_start(out=cos_sb, in_=cos_t[:])
        nc.gpsimd.dma_start(out=sin_sb, in_=sin_t[:])
        nc.gpsimd.dma_start(out=ones_sb, in_=ones_in[:])
        nc.gpsimd.dma_start(out=ident_sb, in_=ident_in[:])
        nc.gpsimd.dma_start(out=rot_sb, in_=rot_t[:])
        nc.gpsimd.dma_start(out=masks_sb, in_=masks_in[:])
        nc.gpsimd.collective_compute(
            "AllGather",
            mybir.AluOpType.bypass,
            replica_groups=[list(range(NCORES))],
            ins=[warm_in[:]],
            outs=[warm_out[:]],
        )
        kT = qkv_sb[:, GROUP, :]

        hs_tiles = {}
        hs_tiles[0] = hs_pool.tile([128, KC, 512], BF16, name="hs_nb")
        for lo, hi in zip(
            [0, 1, 2, 4, 8, 16, 24], [1, 2, 4, 8, 16, 24, 32]
        ):
            nc.sync.dma_start(
                out=hs_tiles[0][:, lo:hi, :], in_=hs_v[:, lo:hi, 0:512]
            )

        def fire_ag(c):
            # seq-chunked AllGather (overlaps all remaining compute)
            nc.gpsimd.collective_compute(
                "AllGather",
                mybir.AluOpType.bypass,
                replica_groups=[list(range(NCORES))],
                ins=[ag_ins[c][:]],
                outs=[ag_outs[c][:]],
            )

        def attn_chunk(c):
            for qi in range(c * 4, c * 4 + 4):
                rhs_q = qkv_sb[:, 0:GROUP, qi * 128 : (qi + 1) * 128]
                njt = qi + 1
                l_ps = psLO.tile([128, 512], F32, tag="l", name="l_ps")
                o_ps = psLO.tile([128, 512], F32, tag="o", name="o_ps")

                def emit_lo(j, pt):
                    nc.tensor.matmul(
                        l_ps,
                        lhsT=ones_sb,
                        rhs=pt,
                        start=(j == 0),
                        stop=(j == njt - 1),
                    )
                    nc.tensor.matmul(
                        o_ps,
                        lhsT=v_nat[:, j, :],
                        rhs=pt,
                        start=(j == 0),
                        stop=(j == njt - 1),
                    )

                def emit_st(stph, j):
                    diag = j == qi
                    nc.tensor.matmul(
                        stph,
                        lhsT=kT[:, j * 128 : (j + 1) * 128],
                        rhs=rhs_q,
                        start=True,
                        stop=not diag,
                    )
                    if diag:  # add -600 above the in-block diagonal
                        nc.tensor.matmul(
                            stph,
                            lhsT=ident_sb,
                            rhs=masks_sb,
                            start=False,
                            stop=True,
                        )

                pend = []
                for j in range(njt):
                    st = psST.tile([128, 512], F32, tag="st", name="st")
                    emit_st(st, j)
                    pt = pt_pool.tile([128, 512], BF16, name="pt")
                    nc.scalar.activation(
                        out=pt,
                        in_=st,
                        func=mybir.ActivationFunctionType.Exp,
                        scale=SCALE,
                    )
                    pend.append((j, pt))
                    if len(pend) > 2:
                        emit_lo(*pend.pop(0))
                for j, pt in pend:
                    emit_lo(j, pt)

                osb = attn_pool.tile([128, 512], F32, tag="osb", name="osb")
                nc.scalar.activation(
                    out=osb, in_=o_ps, func=mybir.ActivationFunctionType.Copy
                )
                linv = attn_pool.tile([128, 512], F32, tag="linv", name="linv")
                nc.vector.reciprocal_approx_fast(linv, l_ps)
                at = attn_pool.tile([128, 512], BF16, tag="at", name="at", bufs=4)
                nc.vector.tensor_mul(at, osb, linv)
                qsub = qi % 4
                nc.sync.dma_start(
                    out=ag_wr[c][:, :, qsub * 128 : (qsub + 1) * 128], in_=at
                )





        for nb in range(NB):
            sl = slice(nb * 512, (nb + 1) * 512)

            # ---------- phase A: qkv_t[:, :, nb] = wa_shard @ hs[nb].T ----
            # (hs for this nb was prefetched; prefetch nb+1 ahead of the
            # vT transposes so the sync FIFO can't head-of-line block it)
            hs_nb = hs_tiles.pop(nb)

            def prefetch_next():
                if nb + 1 < NB and nb + 1 not in hs_tiles:
                    nxt = hs_tiles[nb + 1] = hs_pool.tile(
                        [128, KC, 512], BF16, name="hs_nb"
                    )
                    for lo, hi in zip([0, 8, 16, 24], [8, 16, 24, 32]):
                        nc.sync.dma_start(
                            out=nxt[:, lo:hi, :],
                            in_=hs_v[:, lo:hi, (nb + 1) * 512 : (nb + 2) * 512],
                        )

            # nb0's startup window is HBM-saturated: defer the nb1 prefetch
            # until after the v transposes; later nbs prefetch immediately
            if nb > 0:
                prefetch_next()
            def rope(t):
                x = qkv_sb[:, t, sl]
                rp = psST.tile([128, 512], F32, tag="st", name="rp")
                nc.tensor.matmul(rp, lhsT=rot_sb, rhs=x, start=True, stop=True)
                rs = rope_pool.tile([128, 512], BF16, name="rs")
                nc.vector.tensor_mul(rs, rp, sin_sb[:, sl])
                nc.vector.tensor_mul(x, x, cos_sb[:, sl])
                nc.vector.tensor_add(x, x, rs)

            # k/v first so the v transposes + k rope are ready long before
            # this block's AllGather window can starve the XBAR DMA path
            for g, ms in enumerate(([3, 4, 5], [0, 1, 2])):
                psums = [
                    psA.tile([128, 512], F32, tag=f"a{i}", name=f"psA{i}")
                    for i in range(3)
                ]
                for k in range(KC):
                    for i, m in enumerate(ms):
                        nc.tensor.matmul(
                            psums[i],
                            lhsT=wa_sb[:, k, m * 128 : (m + 1) * 128],
                            rhs=hs_nb[:, k, :],
                            start=(k == 0),
                            stop=(k == KC - 1),
                        )
                for i, m in enumerate(ms):
                    nc.vector.tensor_copy(out=qkv_sb[:, m, sl], in_=psums[i])
                if g == 0:
                    for jj in range(4):
                        j = nb * 4 + jj
                        nc.sync.dma_start_transpose(
                            out=v_nat[:, j, :],
                            in_=qkv_sb[:, GROUP + 1, j * 128 : (j + 1) * 128],
                        )
                    rope(4)
                    rope(3)
                    if nb == 0:
                        prefetch_next()
                    if nb >= 1:
                        fire_ag(nb - 1)
                else:
                    for t in (0, 1, 2):
                        rope(t)

            attn_chunk(nb)

        fire_ag(NB - 1)

        psA.release()
        hs_pool.release()
        # ---------- c_proj: y[mt] = attnT[:, mt].T @ wp_shard ----------
        with (
            tc.tile_pool(name="wp", bufs=1) as wp_pool,
            tc.tile_pool(name="lh", bufs=3) as lh_pool,
            tc.tile_pool(name="ysb", bufs=3) as y_pool,
            tc.tile_pool(name="psC", bufs=2, space="PSUM") as psC,
        ):
            wp_sb = wp_pool.tile([128, KC, P_SHARD], BF16)  # 32KB/part
            for kk in range(0, KC, 8):
                nc.sync.dma_start(
                    out=wp_sb[:, kk : kk + 8, :], in_=wp_v[:, kk : kk + 8, :]
                )
            for c in range(NB):
                for sub in range(4):
                    mt = c * 4 + sub
                    lh = lh_pool.tile([128, KC, 128], BF16, name="lh")
                    nc.sync.dma_start(
                        out=lh, in_=ag_rd[c][:, :, sub * 128 : (sub + 1) * 128]
                    )
                    yp = psC.tile([128, 512], F32, name="yp")
                    for k in range(KC):
                        nc.tensor.matmul(
                            yp,
                            lhsT=lh[:, k, :],
                            rhs=wp_sb[:, k, :],
                            start=(k == 0),
                            stop=(k == KC - 1),
                        )
                    ysb = y_pool.tile([128, P_SHARD], F32, name="ysb")
                    nc.scalar.activation(
                        out=ysb,
                        in_=yp,
                        func=mybir.ActivationFunctionType.Copy,
                    )
                    nc.sync.dma_start(
                        out=y_out[mt * 128 : (mt + 1) * 128, :], in_=ysb
                    )

        for p in (
            psLO,
            psST,
            attn_pool,
            pt_pool,
            rope_pool,
            vnat_pool,
            const_pool,
            w_pool,
            qkv_pool,
        ):
            p.release()

    nc.compile()
    return nc


_CACHED = {}


def _get_module():
    if "nc" not in _CACHED:
        _CACHED["nc"] = build_module()
    return _CACHED["nc"]


def make_in_maps(hidden_states, w_attn, w_proj, rope_cos, rope_sin):
    hidden_states = np.asarray(hidden_states, dtype=np.float32)
    w_attn = np.asarray(w_attn, dtype=np.float32)
    w_proj = np.asarray(w_proj, dtype=np.float32)
    rope_cos = np.asarray(rope_cos, dtype=np.float32)
    rope_sin = np.asarray(rope_sin, dtype=np.float32)

    hs_t = np.ascontiguousarray(hidden_states.reshape(S, H).T).astype(BF16NP)
    cos_t = np.ascontiguousarray(rope_cos.T).astype(BF16NP)
    sin_t = np.ascontiguousarray(rope_sin.T).astype(BF16NP)

    # rotate-half as a matmul: rot(x) = R @ x for x in [HD, S] layout,
    # rot_t = R.T so that lhsT.T @ x = R @ x
    rot_t = np.zeros((HD, HD), dtype=np.float32)
    half = HD // 2
    rot_t[half + np.arange(half), np.arange(half)] = -1.0
    rot_t[np.arange(half), half + np.arange(half)] = 1.0
    rot_t = rot_t.astype(BF16NP)

    # additive causal mask for the diagonal 128x128 block, repeated for
    # the 4 fused heads: masks[k, h*128+qq] = MASKBIG iff qq < k
    kk_, qq_ = np.meshgrid(np.arange(128), np.arange(128), indexing="ij")
    m128 = np.where(qq_ < kk_, MASKBIG, 0.0).astype(np.float32)
    masks = np.tile(m128, (1, GROUP)).astype(BF16NP)

    ones = np.ones((128, 128), dtype=np.float32).astype(BF16NP)
    ident = np.eye(128, dtype=np.float32).astype(BF16NP)

    in_maps = []
    for i in range(NCORES):
        wa_sh = w_attn[i * M_SHARD : (i + 1) * M_SHARD, :]
        wp_sh = w_proj[i * P_SHARD : (i + 1) * P_SHARD, :]
        in_maps.append(
            {
                "hs_t": hs_t,
                "wa_t": np.ascontiguousarray(wa_sh.T).astype(BF16NP),
                "wp_t": np.ascontiguousarray(wp_sh.T).astype(BF16NP),
                "cos_t": cos_t,
                "sin_t": sin_t,
                "rot_t": rot_t,
                "masks_in": masks,
                "ones_in": ones,
                "ident_in": ident,
            }
        )
    return in_maps


def kernel(hidden_states, w_attn, w_proj, rope_cos, rope_sin, **_unused):
    nc = _get_module()
    in_maps = make_in_maps(hidden_states, w_attn, w_proj, rope_cos, rope_sin)
    res = run_bass_kernel_spmd(nc, in_maps, core_ids=list(range(NCORES)))

    out = np.empty((S, H), dtype=np.float32)
    for i in range(NCORES):
        out[:, i * P_SHARD : (i + 1) * P_SHARD] = res.results[i]["y"]
    return out.reshape(B, S, H)


# revision 20
# speedup vs baseline: 1.0211x; 1.0211x over previous
"""Trainium2 Bass kernel for fused causal GQA attention block.

Reference computation (B=1, S=2048, H=4096, NH=32, NKV=8, HD=128):
    qkv = hs @ w_attn.T; rope(q), rope(k); causal GQA attention;
    out @ w_proj.T

Sharding (8 cores, tensor parallel): core i owns kv-group i = rows
[i*768, (i+1)*768) of w_attn (4 q heads + 1 k + 1 v head) and rows
[i*512, (i+1)*512) of w_proj.

All heavy compute runs in bf16 (fp32 PSUM accumulation): full-rate PE
with fast weight load, half the DMA/SBUF/collective traffic of fp32.

Schedule: for each 512-seq block nb: hs resident in SBUF; QKV GEMM in
two weight-group passes with k/v computed FIRST so the V transposes
(XBAR DMA) and k-rope are ready well before any collective can starve
the SDMA engines -> rope(q) -> attention chunk nb (4 q-blocks of 128,
all 4 heads fused into the 512-wide free dim sharing K/V; causal mask
added into PSUM by an extra matmul; exp on ACT software-pipelined 2
deep against the score matmuls; softmax denominator via a ones-matmul,
fast approximate reciprocal on DVE). Each block's bf16 AllGather fires
after the NEXT block's V transposes to dodge XBAR/collective
contention; a tiny warmup AllGather absorbs first-collective latency
and inter-core launch skew. c_proj consumes gathered chunks at the
end, covering the last AllGather. DMA queues are specialized (sync:
hs/vT/at/wp/lh/y HWDGE; scalar: wa + exp; gpsimd: consts + wa half +
collectives) to avoid FIFO head-of-line blocking across streams with
different dependency depths.
"""

import sys

sys.path.insert(0, "/opt/trn_rl_repo")

import ml_dtypes
import numpy as np

import concourse.bass as bass
import concourse.tile as tile
from concourse import bacc, mybir
from concourse.bass_utils import run_bass_kernel_spmd

F32 = mybir.dt.float32
BF16 = mybir.dt.bfloat16
BF16NP = ml_dtypes.bfloat16

B, S, H = 1, 2048, 4096
NH, NKV, HD = 32, 8, 128
GROUP = NH // NKV  # 4
SCALE = 0.08838834764831845
NCORES = 8

M_SHARD = (GROUP + 2) * HD  # 768 rows of w_attn per core
P_SHARD = H // NCORES  # 512 rows of w_proj per core

KC = H // 128  # 32 contraction chunks of the model dim
NB = S // 512  # 4 seq blocks of 512
MT = M_SHARD // 128  # 6 row tiles of qkv_t
QT = S // 128  # 16 q blocks of 128
MASKBIG = -600.0  # additive causal mask (-600 * SCALE ~ -53 before exp)


def build_module() -> bass.Bass:
    nc = bacc.Bacc(
        "TRN2",
        target_bir_lowering=False,
        debug=False,
        num_devices=NCORES,
    )

    hs_t = nc.dram_tensor("hs_t", [H, S], BF16, kind="ExternalInput")
    wa_t = nc.dram_tensor("wa_t", [H, M_SHARD], BF16, kind="ExternalInput")
    wp_t = nc.dram_tensor("wp_t", [H, P_SHARD], BF16, kind="ExternalInput")
    cos_t = nc.dram_tensor("cos_t", [HD, S], BF16, kind="ExternalInput")
    sin_t = nc.dram_tensor("sin_t", [HD, S], BF16, kind="ExternalInput")
    rot_t = nc.dram_tensor("rot_t", [HD, HD], BF16, kind="ExternalInput")
    masks_in = nc.dram_tensor("masks_in", [128, 512], BF16, kind="ExternalInput")
    ones_in = nc.dram_tensor("ones_in", [128, 128], BF16, kind="ExternalInput")
    ident_in = nc.dram_tensor("ident_in", [128, 128], BF16, kind="ExternalInput")
    y_out = nc.dram_tensor("y", [S, P_SHARD], F32, kind="ExternalOutput")

    warm_in = nc.dram_tensor("warm_in", [1, 64], BF16, kind="Internal")
    warm_out = nc.dram_tensor(
        "warm_out", [8, 64], BF16, kind="Internal", addr_space="Shared"
    )
    # per-seq-chunk collective buffers (bf16 halves the wire bytes)
    ag_ins = [
        nc.dram_tensor(f"ag_in{i}", [GROUP * HD, 512], BF16, kind="Internal")
        for i in range(NB)
    ]
    ag_outs = [
        nc.dram_tensor(
            f"ag_out{i}", [H, 512], BF16, kind="Internal", addr_space="Shared"
        )
        for i in range(NB)
    ]

    # DRAM views with 128-partition tiling of the contraction axis
    hs_v = hs_t[:].rearrange("(ko p) n -> p ko n", p=128)  # [128, 32, 2048]
    wa_v = wa_t[:].rearrange("(ko p) m -> p ko m", p=128)  # [128, 32, 768]
    wp_v = wp_t[:].rearrange("(ko p) m -> p ko m", p=128)  # [128, 32, 512]
    ag_rd = [a[:].rearrange("(ko p) n -> p ko n", p=128) for a in ag_outs]
    # write view: feature row h*128+d <- at[d (part), (h, qq)]
    ag_wr = [a[:].rearrange("(h d) s -> d h s", h=GROUP) for a in ag_ins]

    with tile.TileContext(nc) as tc:
        # ---------- persistent pools ----------
        qkv_pool = tc.alloc_tile_pool(name="qkv", bufs=1)
        w_pool = tc.alloc_tile_pool(name="w", bufs=1)
        const_pool = tc.alloc_tile_pool(name="consts", bufs=1)
        vnat_pool = tc.alloc_tile_pool(name="vnat", bufs=1)
        rope_pool = tc.alloc_tile_pool(name="rope", bufs=2)
        pt_pool = tc.alloc_tile_pool(name="pt", bufs=4)
        attn_pool = tc.alloc_tile_pool(name="attn", bufs=3)
        psST = tc.alloc_tile_pool(name="psST", bufs=3, space="PSUM")
        psLO = tc.alloc_tile_pool(name="psLO", bufs=1, space="PSUM")
        hs_pool = tc.alloc_tile_pool(name="hs", bufs=2)
        psA = tc.alloc_tile_pool(name="psA", bufs=1, space="PSUM")

        qkv_sb = qkv_pool.tile([128, MT, S], BF16)  # 24KB/part
        wa_sb = w_pool.tile([128, KC, M_SHARD], BF16)  # 48KB/part
        v_nat = vnat_pool.tile([128, QT, HD], BF16)  # 4KB/part

        ones_sb = const_pool.tile([128, 128], BF16, tag="ones")
        ident_sb = const_pool.tile([128, 128], BF16, tag="ident")
        rot_sb = const_pool.tile([128, HD], BF16, tag="rot")
        masks_sb = const_pool.tile([128, 512], BF16, tag="masks")
        cos_sb = const_pool.tile([128, S], BF16, tag="cos")
        sin_sb = const_pool.tile([128, S], BF16, tag="sin")

        # ---------- preloads ----------
        # wa group-0 (k/v/q3 = cols 384:768) fine-split on the fast scalar
        # path; group-1 (cols 0:384) early on gpsimd
        mc0, mc1 = slice(384, 768), slice(0, 384)
        for lo, hi in zip(
            [0, 1, 2, 4, 8, 16, 24], [1, 2, 4, 8, 16, 24, 32]
        ):
            nc.scalar.dma_start(
                out=wa_sb[:, lo:hi, mc0], in_=wa_v[:, lo:hi, mc0]
            )
        for lo, hi in zip([0, 2, 4, 8, 16, 24], [2, 4, 8, 16, 24, 32]):
            nc.gpsimd.dma_start(
                out=wa_sb[:, lo:hi, mc1], in_=wa_v[:, lo:hi, mc1]
            )
        nc.gpsimd.dma_start(out=cos_sb, in_=cos_t[:])
        nc.gpsimd.dma_start(out=sin_sb, in_=sin_t[:])
        nc.gpsimd.dma_start(out=ones_sb, in_=ones_in[:])
        nc.gpsimd.dma_start(out=ident_sb, in_=ident_in[:])
        nc.gpsimd.dma_start(out=rot_sb, in_=rot_t[:])
        nc.gpsimd.dma_start(out=masks_sb, in_=masks_in[:])
        nc.gpsimd.collective_compute(
            "AllGather",
            mybir.AluOpType.bypass,
            replica_groups=[list(range(NCORES))],
            ins=[warm_in[:]],
            outs=[warm_out[:]],
        )
        kT = qkv_sb[:, GROUP, :]

        hs_tiles = {}
        hs_tiles[0] = hs_pool.tile([128, KC, 512], BF16, name="hs_nb")
        for lo, hi in zip(
            [0, 1, 2, 4, 8, 16, 24], [1, 2, 4, 8, 16, 24, 32]
        ):
            nc.sync.dma_start(
                out=hs_tiles[0][:, lo:hi, :], in_=hs_v[:, lo:hi, 0:512]
            )

        def fire_ag(c):
            # seq-chunked AllGather (overlaps all remaining compute)
            nc.gpsimd.collective_compute(
                "AllGather",
                mybir.AluOpType.bypass,
                replica_groups=[list(range(NCORES))],
                ins=[ag_ins[c][:]],
                outs=[ag_outs[c][:]],
            )

        def attn_chunk(c):
            for qi in range(c * 4, c * 4 + 4):
                rhs_q = qkv_sb[:, 0:GROUP, qi * 128 : (qi + 1) * 128]
                njt = qi + 1
                l_ps = psLO.tile([128, 512], F32, tag="l", name="l_ps")
                o_ps = psLO.tile([128, 512], F32, tag="o", name="o_ps")

                def emit_lo(j, pt):
                    nc.tensor.matmul(
                        l_ps,
                        lhsT=ones_sb,
                        rhs=pt,
                        start=(j == 0),
                        stop=(j == njt - 1),
                    )
                    nc.tensor.matmul(
                        o_ps,
                        lhsT=v_nat[:, j, :],
                        rhs=pt,
                        start=(j == 0),
                        stop=(j == njt - 1),
                    )

                def emit_st(stph, j):
                    diag = j == qi
                    nc.tensor.matmul(
                        stph,
                        lhsT=kT[:, j * 128 : (j + 1) * 128],
                        rhs=rhs_q,
                        start=True,
                        stop=not diag,
                    )
                    if diag:  # add -600 above the in-block diagonal
                        nc.tensor.matmul(
                            stph,
                            lhsT=ident_sb,
                            rhs=masks_sb,
                            start=False,
                            stop=True,
                        )

                pend = []
                for j in range(njt):
                    st = psST.tile([128, 512], F32, tag="st", name="st")
                    emit_st(st, j)
                    pt = pt_pool.tile([128, 512], BF16, name="pt")
                    nc.scalar.activation(
                        out=pt,
                        in_=st,
                        func=mybir.ActivationFunctionType.Exp,
                        scale=SCALE,
                    )
                    pend.append((j, pt))
                    if len(pend) > 2:
                        emit_lo(*pend.pop(0))
                for j, pt in pend:
                    emit_lo(j, pt)

                osb = attn_pool.tile([128, 512], F32, tag="osb", name="osb")
                nc.scalar.activation(
                    out=osb, in_=o_ps, func=mybir.ActivationFunctionType.Copy
                )
                linv = attn_pool.tile([128, 512], F32, tag="linv", name="linv")
                nc.vector.reciprocal_approx_fast(linv, l_ps)
                at = attn_pool.tile([128, 512], BF16, tag="at", name="at", bufs=4)
                nc.vector.tensor_mul(at, osb, linv)
                qsub = qi % 4
                nc.sync.dma_start(
                    out=ag_wr[c][:, :, qsub * 128 : (qsub + 1) * 128], in_=at
                )





        for nb in range(NB):
            sl = slice(nb * 512, (nb + 1) * 512)

            # ---------- phase A: qkv_t[:, :, nb] = wa_shard @ hs[nb].T ----
            # (hs for this nb was prefetched; prefetch nb+1 ahead of the
            # vT transposes so the sync FIFO can't head-of-line block it)
            hs_nb = hs_tiles.pop(nb)

            def prefetch_next():
                if nb + 1 < NB and nb + 1 not in hs_tiles:
                    nxt = hs_tiles[nb + 1] = hs_pool.tile(
                        [128, KC, 512], BF16, name="hs_nb"
                    )
                    for lo, hi in zip([0, 8, 16, 24], [8, 16, 24, 32]):
                        nc.sync.dma_start(
                            out=nxt[:, lo:hi, :],
                            in_=hs_v[:, lo:hi, (nb + 1) * 512 : (nb + 2) * 512],
                        )

            # nb0's startup window is HBM-saturated: defer the nb1 prefetch
            # until after the v transposes; later nbs prefetch immediately
            if nb > 0:
                prefetch_next()
            def rope(t):
                x = qkv_sb[:, t, sl]
                rp = psST.tile([128, 512], F32, tag="st", name="rp")
                nc.tensor.matmul(rp, lhsT=rot_sb, rhs=x, start=True, stop=True)
                rs = rope_pool.tile([128, 512], BF16, name="rs")
                nc.vector.tensor_mul(rs, rp, sin_sb[:, sl])
                nc.vector.tensor_mul(x, x, cos_sb[:, sl])
                nc.vector.tensor_add(x, x, rs)

            # k/v first so the v transposes + k rope are ready long before
            # this block's AllGather window can starve the SDMA engines.
            # nb0: single 6-wide pass (attention pools are still idle, so
            # borrow the st pool's banks) -- halves the required startup
            # DMA stream rate vs two passes.
            if nb == 0:
                psums = {}
                for i, m in enumerate((3, 4, 5)):
                    psums[m] = psA.tile(
                        [128, 512], F32, tag=f"a{i}", name=f"psA{i}"
                    )
                for m in (0, 1, 2):
                    psums[m] = psST.tile([128, 512], F32, tag="st", name="st")
                for k in range(KC):
                    for m in (3, 4, 5, 0, 1, 2):
                        nc.tensor.matmul(
                            psums[m],
                            lhsT=wa_sb[:, k, m * 128 : (m + 1) * 128],
                            rhs=hs_nb[:, k, :],
                            start=(k == 0),
                            stop=(k == KC - 1),
                        )
                for m in (3, 4, 5, 0, 1, 2):
                    nc.vector.tensor_copy(out=qkv_sb[:, m, sl], in_=psums[m])
                for jj in range(4):
                    nc.sync.dma_start_transpose(
                        out=v_nat[:, jj, :],
                        in_=qkv_sb[:, GROUP + 1, jj * 128 : (jj + 1) * 128],
                    )
                for t in (4, 3, 0, 1, 2):
                    rope(t)
                prefetch_next()
            else:
                for g, ms in enumerate(([3, 4, 5], [0, 1, 2])):
                    psums = [
                        psA.tile([128, 512], F32, tag=f"a{i}", name=f"psA{i}")
                        for i in range(3)
                    ]
                    for k in range(KC):
                        for i, m in enumerate(ms):
                            nc.tensor.matmul(
                                psums[i],
                                lhsT=wa_sb[:, k, m * 128 : (m + 1) * 128],
                                rhs=hs_nb[:, k, :],
                                start=(k == 0),
                                stop=(k == KC - 1),
                            )
                    for i, m in enumerate(ms):
                        nc.vector.tensor_copy(
                            out=qkv_sb[:, m, sl], in_=psums[i]
                        )
                    if g == 0:
                        for jj in range(4):
                            j = nb * 4 + jj
                            nc.sync.dma_start_transpose(
                                out=v_nat[:, j, :],
                                in_=qkv_sb[
                                    :, GROUP + 1, j * 128 : (j + 1) * 128
                                ],
                            )
                        rope(4)
                        rope(3)
                        fire_ag(nb - 1)
                    else:
                        for t in (0, 1, 2):
                            rope(t)

            attn_chunk(nb)

        fire_ag(NB - 1)

        psA.release()
        hs_pool.release()
        # ---------- c_proj: y[mt] = attnT[:, mt].T @ wp_shard ----------
        with (
            tc.tile_pool(name="wp", bufs=1) as wp_pool,
            tc.tile_pool(name="lh", bufs=3) as lh_pool,
            tc.tile_pool(name="ysb", bufs=3) as y_pool,
            tc.tile_pool(name="psC", bufs=2, space="PSUM") as psC,
        ):
            wp_sb = wp_pool.tile([128, KC, P_SHARD], BF16)  # 32KB/part
            for kk in range(0, KC, 8):
                nc.sync.dma_start(
                    out=wp_sb[:, kk : kk + 8, :], in_=wp_v[:, kk : kk + 8, :]
                )
            for c in range(NB):
                for sub in range(4):
                    mt = c * 4 + sub
                    lh = lh_pool.tile([128, KC, 128], BF16, name="lh")
                    nc.sync.dma_start(
                        out=lh, in_=ag_rd[c][:, :, sub * 128 : (sub + 1) * 128]
                    )
                    yp = psC.tile([128, 512], F32, name="yp")
                    for k in range(KC):
                        nc.tensor.matmul(
                            yp,
                            lhsT=lh[:, k, :],
                            rhs=wp_sb[:, k, :],
                            start=(k == 0),
                            stop=(k == KC - 1),
                        )
                    ysb = y_pool.tile([128, P_SHARD], F32, name="ysb")
                    nc.scalar.activation(
                        out=ysb,
                        in_=yp,
                        func=mybir.ActivationFunctionType.Copy,
                    )
                    nc.sync.dma_start(
                        out=y_out[mt * 128 : (mt + 1) * 128, :], in_=ysb
                    )

        for p in (
            psLO,
            psST,
            attn_pool,
            pt_pool,
            rope_pool,
            vnat_pool,
            const_pool,
            w_pool,
            qkv_pool,
        ):
            p.release()

    nc.compile()
    return nc


_CACHED = {}


def _get_module():
    if "nc" not in _CACHED:
        _CACHED["nc"] = build_module()
    return _CACHED["nc"]


def make_in_maps(hidden_states, w_attn, w_proj, rope_cos, rope_sin):
    hidden_states = np.asarray(hidden_states, dtype=np.float32)
    w_attn = np.asarray(w_attn, dtype=np.float32)
    w_proj = np.asarray(w_proj, dtype=np.float32)
    rope_cos = np.asarray(rope_cos, dtype=np.float32)
    rope_sin = np.asarray(rope_sin, dtype=np.float32)

    hs_t = np.ascontiguousarray(hidden_states.reshape(S, H).T).astype(BF16NP)
    cos_t = np.ascontiguousarray(rope_cos.T).astype(BF16NP)
    sin_t = np.ascontiguousarray(rope_sin.T).astype(BF16NP)

    # rotate-half as a matmul: rot(x) = R @ x for x in [HD, S] layout,
    # rot_t = R.T so that lhsT.T @ x = R @ x
    rot_t = np.zeros((HD, HD), dtype=np.float32)
    half = HD // 2
    rot_t[half + np.arange(half), np.arange(half)] = -1.0
    rot_t[np.arange(half), half + np.arange(half)] = 1.0
    rot_t = rot_t.astype(BF16NP)

    # additive causal mask for the diagonal 128x128 block, repeated for
    # the 4 fused heads: masks[k, h*128+qq] = MASKBIG iff qq < k
    kk_, qq_ = np.meshgrid(np.arange(128), np.arange(128), indexing="ij")
    m128 = np.where(qq_ < kk_, MASKBIG, 0.0).astype(np.float32)
    masks = np.tile(m128, (1, GROUP)).astype(BF16NP)

    ones = np.ones((128, 128), dtype=np.float32).astype(BF16NP)
    ident = np.eye(128, dtype=np.float32).astype(BF16NP)

    in_maps = []
    for i in range(NCORES):
        wa_sh = w_attn[i * M_SHARD : (i + 1) * M_SHARD, :]
        wp_sh = w_proj[i * P_SHARD : (i + 1) * P_SHARD, :]
        in_maps.append(
            {
                "hs_t": hs_t,
                "wa_t": np.ascontiguousarray(wa_sh.T).astype(BF16NP),
                "wp_t": np.ascontiguousarray(wp_sh.T).astype(BF16NP),
                "cos_t": cos_t,
                "sin_t": sin_t,
                "rot_t": rot_t,
                "masks_in": masks,
                "ones_in": ones,
                "ident_in": ident,
            }
        )
    return in_maps


def kernel(hidden_states, w_attn, w_proj, rope_cos, rope_sin, **_unused):
    nc = _get_module()
    in_maps = make_in_maps(hidden_states, w_attn, w_proj, rope_cos, rope_sin)
    res = run_bass_kernel_spmd(nc, in_maps, core_ids=list(range(NCORES)))

    out = np.empty((S, H), dtype=np.float32)
    for i in range(NCORES):
        out[:, i * P_SHARD : (i + 1) * P_SHARD] = res.results[i]["y"]
    return out.reshape(B, S, H)
